# revision 21
# baseline (speedup 1.0000x reference)
"""Causal self-attention (B=2, T=2048, C=1024, H=16, D=64) on 8 NeuronCores.

Sharding: core = (batch b, head-group g); each of the 8 cores handles one
batch and 4 of the 16 heads (data parallel on B, tensor parallel on heads).
Each core computes q/k/v projections for its heads, rope, causal softmax
attention, and a partial out-projection; the host sums the 4 per-batch
partials and adds bout + bqkv_v @ Wout (the v-bias term commutes through
the attention average, so it is applied once on the host).

v8 design (185.5us; v2 baseline was 195.1us):
  - ONE merged exp per k-round over a [P, 2(j), 2(pp), QT] PSUM S tile
    (j = tile_position row half -> bank, pp = head pair -> half bank).
    Halves the ACT instruction count (~350-cycle fixed overhead per
    ACT) and gives the S pool true double buffering (v2 allocated both
    pss bufs every round, serializing S -> exp -> S).
  - O matmuls lag TWO rounds behind S (not one) so they never wait on
    the just-issued exp: the ~1us ACT latency is out of the per-round
    critical cycle. Clean rounds run at ~1.0us (ACT-throughput-paced).
  - x is fully SBUF-resident: pair 0 in fine 512-col chunks (first
    qkproj at ~11us, bounded by the fixed ~7.5us NEFF preamble), pairs
    1-3 in one [P,8,1536] tile split across BOTH DMA queues in the
    prologue. DMA trigger instructions cost ~5ns/descriptor of engine
    time and queues are in-order, so mid-kernel x bursts would delay
    the latency-critical divide DMAs behind megabytes of transfer.
  - steady-state divide: denominator row -> transpose DMA -> [P,8]
    reciprocal -> writeback (latency hidden by the 1-tile pipeline
    slack; tiles 0-3's divides feed late-deferred outprojs so their
    latency is entirely harmless). LAST tile: the round trip sits on
    the tail critical path, so it is done on-engine with two DVE 32x32
    block transposes instead (spread row 64 across partitions 64:96,
    strided 16-elem/lane reciprocal, transpose back).
  - rope runs entirely on DVE (the gpsimd cos-mul made diagonal masks
    queue behind it, stalling O quads ~3us on $S[166]).
  - injected projection work for pair N+1 is paced over the rounds of
    TWO tiles; outproj(12,13) are held back to fill the tail divide
    gap; tail outproj evacuations ride the then-idle ScalarE.
"""
import sys
sys.path.insert(0, '/opt/trn_rl_repo')

import numpy as np
import ml_dtypes
from contextlib import ExitStack

import concourse.bass as bass
import concourse.tile as tile
from concourse import mybir
from concourse.bass_utils import run_bass_kernel_spmd

B, T, C, H, D = 2, 2048, 1024, 16, 64
HPC = 4          # heads per core
G = H // HPC     # head groups (cores per batch)
N_CORES = B * G
SCALE = 1.0 / np.sqrt(D)
P = 128
QT = 256         # q tile width
TT = T // QT     # q tiles (8)
NCC = C // P     # 128-deep contraction chunks (8)
VW = 66          # v cols per head: 64 + ones + pad
F32 = mybir.dt.float32
BF16 = mybir.dt.bfloat16
BF = ml_dtypes.bfloat16


def _tril_mask():
    p = np.arange(P)[:, None]
    f = np.arange(P)[None, :]
    return (p <= f).astype(np.float32)


# walrus in this toolchain can't encode >1 sem wait on one instruction
# ("Too many sync wait commands"); split excess waits onto preceding NoOps.
def _split_waits(nc, maxw=1):
    for f in nc.m.functions:
        for bb in f.blocks:
            out = []
            for inst in bb.instructions:
                si = getattr(inst, 'sync_info', None)
                if si is not None and si.on_wait and len(si.on_wait) > maxw:
                    waits = list(si.on_wait)
                    extra, keep = waits[:-maxw], waits[-maxw:]
                    for i in range(0, len(extra), maxw):
                        out.append(mybir.InstNoOp(
                            name=f"{inst.name}-wsplit{i}",
                            sync_info=mybir.SyncInfo(
                                on_wait=extra[i:i + maxw], on_update=[]),
                            bass_nofuse=True,
                            engine=inst.engine,
                        ))
                    inst.sync_info = mybir.SyncInfo(
                        on_wait=keep, on_update=list(si.on_update or []))
                out.append(inst)
            bb.instructions[:] = out


def build_nc(split=True):
    nc = bass.Bass()
    xT = nc.dram_tensor("xT", [C, T], BF16, kind="ExternalInput")
    wq = nc.dram_tensor("wq", [C, 256], BF16, kind="ExternalInput")
    wk = nc.dram_tensor("wk", [C, 256], BF16, kind="ExternalInput")
    wv = nc.dram_tensor("wv", [C, HPC * VW], BF16, kind="ExternalInput")
    bqk = nc.dram_tensor("bqk", [P, 4], F32, kind="ExternalInput")  # qA qB kA kB
    scs = nc.dram_tensor("scs", [P, 2 * T], BF16, kind="ExternalInput")  # sin|cos
    wout = nc.dram_tensor("wout", [256, C], BF16, kind="ExternalInput")
    y = nc.dram_tensor("y", [T, C], BF16, kind="ExternalOutput")
    masks_d = nc.inline_tensor(_tril_mask(), name="cmasks")

    with tile.TileContext(nc) as tc:
        with ExitStack() as ctx:
            # ---- resident pools ----
            wpool = ctx.enter_context(tc.tile_pool(name="wts", bufs=1))
            qkpool = ctx.enter_context(tc.tile_pool(name="qk", bufs=1))
            vpool = ctx.enter_context(tc.tile_pool(name="v", bufs=1))
            otpool = ctx.enter_context(tc.tile_pool(name="ot", bufs=1))

            bqk_sb = wpool.tile([P, 4], F32, tag="bqk")
            wq_sb = wpool.tile([P, NCC, 256], BF16, tag="wq")
            wk_sb = wpool.tile([P, NCC, 256], BF16, tag="wk")
            wv_sb = wpool.tile([P, NCC, HPC * VW], BF16, tag="wv")
            scs_sb = wpool.tile([P, 2 * T], BF16, tag="scs")
            sin4 = scs_sb[:, 0:T]
            cos4 = scs_sb[:, T:2 * T]
            wout_sb = wpool.tile([P, 2, C], BF16, tag="wout")
            masks_f = wpool.tile([P, P], F32, tag="masksf")
            masks_sb = wpool.tile([P, P], BF16, tag="masks")
            ones_f = wpool.tile([P, 64], F32, tag="onesf")

            # q/k per head pair, rows = [A(h0) B(h0) A(h1) B(h1)] x 32
            # (A/B = rotary low/high halves) so one matmul contracts a
            # whole head (K=64)
            qp0 = qkpool.tile([P, T], BF16, tag="qp0")
            qp1 = qkpool.tile([P, T], BF16, tag="qp1")
            kp0 = qkpool.tile([P, T], BF16, tag="kp0")
            kp1 = qkpool.tile([P, T], BF16, tag="kp1")
            qk_tiles = [qp0, qp1, kp0, kp1]
            qp = [qp0, qp1]
            kp = [kp0, kp1]
            w_of = {0: wq_sb, 1: wq_sb, 2: wk_sb, 3: wk_sb}
            col_of = {0: 0, 1: 128, 2: 0, 3: 128}

            # V tiles [t-block, 4*66] (col 64 per head = ones, 65 = pad)
            NTB = T // P
            v_tiles = [vpool.tile([P, HPC * VW], BF16, tag=f"v{tb}",
                                  name=f"v{tb}")
                       for tb in range(NTB)]

            # O^T in SBUF: heads 0,1 stacked / heads 2,3 stacked
            ot_sb = [otpool.tile([P, T], BF16, tag=f"otsb{i}",
                                 name=f"otsb{i}")
                     for i in range(2)]

            with ExitStack() as stream:
                rtmp = stream.enter_context(tc.tile_pool(name="rtmp", bufs=4))
                # pss tiles are [P, 2(j), 2(pp), QT]: j indexes the PSUM
                # bank so the two concurrent row-group-packed S matmuls
                # (tile_position j=0/1) drain into DIFFERENT banks —
                # concurrent PE drains into one bank are a fatal PSUM
                # write collision on hardware. pp0/pp1 share a bank but
                # those matmuls are serialized (same row group).
                ps_s = stream.enter_context(
                    tc.tile_pool(name="pss", bufs=2, space="PSUM"))
                ps_ot = stream.enter_context(
                    tc.tile_pool(name="psot", bufs=2, space="PSUM"))
                ps_pr = stream.enter_context(
                    tc.tile_pool(name="pspr", bufs=2, space="PSUM"))
                espool = stream.enter_context(tc.tile_pool(name="es", bufs=5))
                dpool = stream.enter_context(tc.tile_pool(name="dv", bufs=2))
                opool = stream.enter_context(tc.tile_pool(name="osb", bufs=2))

                # x pair 0 is loaded in fine 512-col chunks for the
                # fastest possible prologue start; pairs 1-3 live in one
                # resident tile loaded with 8 per-cc DMAs whose DRAM runs
                # are 3KB (descriptor-count, i.e. trigger-engine-time, is
                # 1/3 of per-pair chunked loads), issued once in the
                # prologue so the SP queue has no mid-kernel bursts.
                x0_sb = wpool.tile([P, NCC, 2 * QT], BF16, tag="x0")
                xr_sb = wpool.tile([P, NCC, 3 * 2 * QT], BF16, tag="xr")
                xt = {0: x0_sb}
                for tp in range(1, 4):
                    xt[tp] = xr_sb.rearrange(
                        "p c (u q) -> p c u q", u=3)[:, :, tp - 1, :]

                # q/k projection for one (jb, q-tile-pair): one 8-chunk
                # PSUM generation at N=512; evacuation folds the bias
                # (per-partition) and the bf16 downcast into one DVE op.
                def qkproj(jb, tp):
                    wsb, c0 = w_of[jb], col_of[jb]
                    dst = qk_tiles[jb][:, tp * 2 * QT:(tp + 1) * 2 * QT]
                    ps = ps_pr.tile([P, 512], F32, tag="pspr",
                                    name=f"qk{jb}_{tp}")
                    for cc in range(NCC):
                        nc.tensor.matmul(
                            ps[:], wsb[:, cc, c0:c0 + 128], xt[tp][:, cc, :],
                            start=(cc == 0), stop=(cc == NCC - 1))
                    nc.vector.tensor_scalar_add(dst, ps[:],
                                                bqk_sb[:, jb:jb + 1])

                # rope on the interleaved [A;B]-per-head layout:
                #   t' = t * cosF + swap32(t) * sinF'
                # where swap32 exchanges adjacent 32-partition blocks (DMA)
                # and sinF' = [-sin, +sin, -sin, +sin] (host-built sign).
                def rope(ti, tp):
                    t = qk_tiles[ti]
                    s = slice(tp * 2 * QT, (tp + 1) * 2 * QT)
                    u = rtmp.tile([P, 2 * QT], BF16, tag="ru", name="ru")
                    for blk in range(4):
                        r, rs = 32 * blk, 32 * (blk ^ 1)
                        nc.sync.dma_start(u[r:r + 32, :],
                                          t[rs:rs + 32, s])
                    t1 = rtmp.tile([P, 2 * QT], BF16, tag="r1", name="r1")
                    t2 = rtmp.tile([P, 2 * QT], BF16, tag="r2", name="r2")
                    nc.vector.tensor_mul(t1[:], t[:, s], cos4[:, s])
                    nc.vector.tensor_mul(t2[:], u[:], sin4[:, s])
                    nc.vector.tensor_add(t[:, s], t1[:], t2[:])

                def vproj(tb):
                    ps = ps_pr.tile([P, 512], F32, tag="pspr",
                                    name=f"psv{tb}")[:, 0:HPC * VW]
                    for cc in range(NCC):
                        nc.tensor.matmul(
                            ps, xt[tb // 4][:, cc, (tb % 4) * P:
                                            (tb % 4 + 1) * P],
                            wv_sb[:, cc, :],
                            start=(cc == 0), stop=(cc == NCC - 1))
                    nc.vector.tensor_copy(v_tiles[tb][:], ps)
                    ones_cols = v_tiles[tb].rearrange(
                        "p (h e) -> p h e", e=VW)[:, :, 64]
                    nc.vector.tensor_scalar_add(ones_cols, ones_cols, 1.0)

                # stage 1 of the divide: copy O^T psum (incl. denominator
                # row 64) to SBUF in bf16 — frees the psot banks for the
                # next tt — then the denominator transpose DMA (the [1,512]
                # row must be spread across partitions: DVE reciprocal runs
                # at ~8 cycles/element per lane). x is fully resident after
                # the prologue, so the SP queue has no bursts for these tiny
                # DMAs to queue behind; the last tile rides the then-idle
                # ACT queue.
                def divides_a(tt, ot2, last=False):
                    # The [1,512] denominator row must be spread across
                    # partitions for a cheap reciprocal (DVE recip is ~8
                    # cyc/elem per lane). Steady tiles use a transpose DMA
                    # round-trip (latency hidden by the pipeline); the
                    # LAST tile does it on-engine with two DVE 32x32 block
                    # transposes (~0.6us each vs ~2.5us per DMA leg) since
                    # the round-trip sits on the tail critical path.
                    otf = []
                    if last:
                        rrs = []
                        with nc.allow_low_precision(
                                reason="softmax denom recip in bf16: 0.4% "
                                       "scale error, inside the 2e-2 gate"):
                            for pp in range(2):
                                of = dpool.tile([96, 2, QT], BF16,
                                                tag=f"otfl{pp}",
                                                name=f"otfl{pp}")
                                nc.vector.tensor_copy(of[0:VW], ot2[pp][:])
                                otf.append(of)
                                dnt = dpool.tile([96, 2, QT], BF16,
                                                 tag=f"dnt{pp}", name="dnt")
                                fl = of[64:96, :, :].rearrange(
                                    "p a c -> p (a c)")
                                nc.vector.transpose(
                                    dnt[64:96, :, :].rearrange(
                                        "p a c -> p (a c)"), fl)
                                rw = dpool.tile([96, 2, QT], BF16,
                                                tag=f"rw{pp}", name="rw")
                                nc.vector.reciprocal(
                                    rw[64:96, :, :].rearrange(
                                        "p a c -> p (a c)")[:, 0:512:32],
                                    dnt[64:96, :, :].rearrange(
                                        "p a c -> p (a c)")[:, 0:512:32])
                                rt = dpool.tile([96, 2, QT], BF16,
                                                tag=f"rt{pp}", name="rt")
                                nc.vector.transpose(
                                    rt[64:96, :, :].rearrange(
                                        "p a c -> p (a c)"),
                                    rw[64:96, :, :].rearrange(
                                        "p a c -> p (a c)"))
                                rrs.append(rt[64:65, :, :].rearrange(
                                    "a b c -> a (b c)"))
                        return (otf, rrs, 64)
                    otf = []
                    dn = dpool.tile([P, 8], BF16, tag="dn", name="dn")
                    for pp in range(2):
                        of = dpool.tile([VW, 2, QT], BF16, tag=f"otf{pp}",
                                        name=f"otf{pp}")
                        nc.vector.tensor_copy(of[:], ot2[pp][:])
                        nc.sync.dma_start(
                            dn[:, 4 * pp:4 * pp + 4],
                            of[64:65, :, :].rearrange("a b c -> a (b c)"))
                        otf.append(of)
                    rr = dpool.tile([1, 2, 2, QT], BF16, tag="rr", name="rr")
                    with nc.allow_low_precision(
                            reason="softmax denom recip in bf16: 0.4% scale "
                                   "error, well inside the 2e-2 gate"):
                        dnr = dpool.tile([P, 8], BF16, tag="dnr",
                                         name="dnr")
                        nc.vector.reciprocal(dnr[:], dn[:])
                        for pp in range(2):
                            nc.sync.dma_start(
                                rr[0:1, pp, :, :].rearrange(
                                    "a b c -> a (b c)"),
                                dnr[:, 4 * pp:4 * pp + 4])
                    return (otf,
                            [rr[0:1, pp, :, :].rearrange("a b c -> a (b c)")
                             for pp in range(2)], 0)

                # stage 2: broadcast recip across 64 partitions via PE,
                # then ot_sb[h] = otf[h] * recip (DVE, bf16 2x mode).
                def divides_b(tt, pend):
                    otf, rrs, rbase = pend
                    for pp in range(2):
                        rb = ps_pr.tile([P, 512], F32, tag="pspr",
                                        name=f"rb{pp}")[0:64, :]
                        nc.tensor.matmul(
                            rb, ones_sb[rbase:rbase + 1, :], rrs[pp],
                            start=True, stop=True)
                        # multiply straight from the broadcast PSUM (1x
                        # mode, 392ns) instead of copy-to-bf16 + 2x muls
                        # (1046ns) — DVE is the binding engine in the
                        # injection-heavy tiles
                        rbv = rb.rearrange("p (a b) -> p a b", a=2)
                        for j in range(2):
                            dst = ot_sb[pp][64 * j:64 * j + 64,
                                            tt * QT:(tt + 1) * QT]
                            nc.vector.tensor_mul(
                                dst, otf[pp][0:64, j, :], rbv[:, j, :])

                def outproj(tb, tail=False):
                    o_sb = opool.tile([P, C], BF16, tag="osb", name="osb")
                    for nt in range(2):
                        ps = ps_pr.tile([P, 512], F32, tag="pspr",
                                        name=f"pso{tb}_{nt}")
                        for rc in range(2):
                            nc.tensor.matmul(
                                ps[:], ot_sb[rc][:, tb * P:(tb + 1) * P],
                                wout_sb[:, rc, nt * 512:(nt + 1) * 512],
                                start=(rc == 0), stop=(rc == 1))
                        # after the last exp the ACT engine is idle; use it
                        # for the tail evacuations so DVE isn't the tail
                        # critical path
                        if tail:
                            nc.scalar.copy(
                                o_sb[:, nt * 512:(nt + 1) * 512], ps[:])
                            nc.sync.dma_start(
                                y[tb * P:(tb + 1) * P,
                                  nt * 512:(nt + 1) * 512],
                                o_sb[:, nt * 512:(nt + 1) * 512])
                        else:
                            nc.vector.tensor_copy(
                                o_sb[:, nt * 512:(nt + 1) * 512], ps[:])
                    if not tail:
                        nc.sync.dma_start(y[tb * P:(tb + 1) * P, :], o_sb[:])

                # ---- prologue ----
                # DMA triggers first (ACT queue), ordered by first use;
                # the ACT table-load dummy exp goes AFTER the triggers so
                # it overlaps the transfers instead of delaying them.
                nc.scalar.dma_start(bqk_sb[:], bqk[:])
                for cc in range(NCC):
                    nc.sync.dma_start(x0_sb[:, cc, :],
                                      xT[cc * P:(cc + 1) * P, 0:2 * QT])
                # x pair 1 FIRST on the weights queue: tiles 0-1 finish in
                # ~6us of rounds while pair-1 data takes ~15us to land —
                # pushing the prologue matmuls a few us later is cheaper
                # than a 6us PE bubble (plus the HAM re-throttle it causes)
                # at t~25us.
                for cc in range(NCC):
                    nc.scalar.dma_start(
                        xr_sb[:, cc, 0:2 * QT],
                        xT[cc * P:(cc + 1) * P, 2 * QT:4 * QT])
                for cc in range(NCC):
                    nc.scalar.dma_start(wq_sb[:, cc, :],
                                        wq[cc * P:(cc + 1) * P, :])
                for cc in range(NCC):
                    nc.scalar.dma_start(wk_sb[:, cc, :],
                                        wk[cc * P:(cc + 1) * P, :])
                nc.scalar.dma_start(masks_f[:], masks_d[:])
                # scs slices used by rope(tp=0) first, the rest later
                nc.scalar.dma_start(scs_sb[:, 0:2 * QT], scs[:, 0:2 * QT])
                nc.scalar.dma_start(scs_sb[:, T:T + 2 * QT],
                                    scs[:, T:T + 2 * QT])
                nc.scalar.dma_start(wv_sb[:],
                                    wv.rearrange("(o p) n -> p o n", p=P))

                nc.vector.memset(ones_f[:], 1.0)
                ones_sb = wpool.tile([P, 64], BF16, tag="ones")
                nc.vector.tensor_copy(ones_sb[:], ones_f[:])
                # dummy exp: pulls the ~2.7us ACT table load into the DMA
                # prologue
                nc.scalar.activation(ones_f[0:1, 0:1], ones_f[0:1, 0:1],
                                     mybir.ActivationFunctionType.Exp)
                nc.vector.tensor_copy(masks_sb[:], masks_f[:])

                for jb in range(4):
                    qkproj(jb, 0)
                for ti in (0, 2, 1, 3):
                    rope(ti, 0)
                for tb in range(4):
                    vproj(tb)
                # remaining weights + x pair-1 prefetch, behind the
                # first-use loads on their queues
                nc.scalar.dma_start(scs_sb[:, 2 * QT:T], scs[:, 2 * QT:T])
                nc.scalar.dma_start(scs_sb[:, T + 2 * QT:2 * T],
                                    scs[:, T + 2 * QT:2 * T])
                # x pairs 2-3 (needed ~t=55/90us) ride the ACT queue
                # behind the weights. wout goes last (first outproj isn't
                # until ~tt=4).
                for cc in range(NCC):
                    nc.scalar.dma_start(
                        xr_sb[:, cc, 2 * QT:6 * QT],
                        xT[cc * P:(cc + 1) * P, 4 * QT:T])
                nc.scalar.dma_start(wout_sb[:],
                                    wout.rearrange("(o p) n -> p o n", p=P))

                # ---- streaming attention with injected work ----
                queue = []  # closures of next-tile + prev-tile work
                prev = None
                prev_out = []  # outproj closures of prev tile not yet run
                late = []  # early-tile outprojs deferred into the last tiles
                carry = None  # prev tile's deferred final O flush

                def flush_carry(last=False):
                    # the previous tile's last two O rounds, deferred past
                    # the next tile's first S so they never wait on exp
                    nonlocal carry, prev, prev_out
                    ptt, pot2, pfn, popend = carry
                    carry = None
                    for i, pnd in enumerate(popend):
                        pfn(*pnd, stop=(i == len(popend) - 1))
                    if prev is not None:
                        divides_b(*prev)
                        prev = None
                    if last:
                        pend = divides_a(ptt, pot2, last=True)
                        for tb in (2 * ptt - 2, 2 * ptt - 1):
                            outproj(tb, tail=True)
                        prev_out = [lambda tb=tb: outproj(tb, tail=True)
                                    for tb in range(2 * ptt, 2 * ptt + 2)]
                        prev = (ptt, pend)
                        return
                    for fn in prev_out:
                        fn()
                    outs = [lambda tb=tb, tl=last: outproj(tb, tail=tl)
                            for tb in range(2 * ptt, 2 * ptt + 2)]
                    if ptt < 4:
                        late.extend(outs)
                        prev_out = []
                    else:
                        prev_out = outs
                    prev = (ptt, divides_a(ptt, pot2, last=last))

                for tt in range(TT):
                    nk = 2 * tt + 2
                    if tt % 2 == 0 and tt // 2 + 1 < TT // 2:
                        ntp = tt // 2 + 1
                        queue += [lambda jb=jb, t=ntp: qkproj(jb, t)
                                  for jb in range(4)]
                        queue += [lambda ti=ti, t=ntp: rope(ti, t)
                                  for ti in (0, 2, 1, 3)]
                        queue += [lambda tb=tb: vproj(tb)
                                  for tb in range(4 * ntp, 4 * ntp + 4)]
                    ot2 = [ps_ot.tile([VW, 2, QT], F32, tag="psot",
                                      name=f"psot{pp}") for pp in range(2)]
                    opend = []  # rounds whose O quad hasn't been emitted

                    def o_quad(pes, poff, pk, stop=False, ot2=ot2):
                        for pp in range(2):
                            for j in range(2):
                                h = 2 * pp + j
                                nc.tensor.matmul(
                                    ot2[pp][:, j, poff:],
                                    v_tiles[pk][:, VW * h:VW * h + VW],
                                    pes[:, j, pp, poff:],
                                    start=(pk == 0 and j == 0),
                                    stop=(stop and j == 1))

                    for kblk in range(nk):
                        off = max(0, (kblk - 2 * tt)) * P
                        ks = slice(kblk * P, (kblk + 1) * P)
                        qs = slice(tt * QT + off, (tt + 1) * QT)
                        s4 = ps_s.tile([P, 2, 2, QT], F32, tag="pss",
                                       name="pss")
                        for pp in range(2):
                            for j in range(2):
                                hs = slice(64 * j, 64 * j + 64)
                                nc.tensor.matmul(
                                    s4[:, j, pp, off:], kp[pp][hs, ks],
                                    qp[pp][hs, qs],
                                    start=True, stop=True,
                                    tile_position=(64 * j, 0))
                        es4 = espool.tile([P, 2, 2, QT], BF16, tag="es",
                                          name="es")
                        nc.scalar.activation(
                            es4[:, :, :, off:], s4[:, :, :, off:],
                            mybir.ActivationFunctionType.Exp, scale=SCALE)
                        if kblk >= 2 * tt:
                            for pp in range(2):
                                nc.gpsimd.tensor_mul(
                                    es4[:, :, pp, off:off + P],
                                    es4[:, :, pp, off:off + P],
                                    masks_sb[:, None, :].to_broadcast(
                                        (P, 2, P)))
                        # O lags TWO rounds behind S so it never waits on
                        # the just-issued exp (the 1-round lag put exp's
                        # ~1us latency inside the per-round critical cycle)
                        opend.append((es4, off, kblk))
                        if len(opend) > 2:
                            o_quad(*opend.pop(0))
                        # deferred prev-tile flush right after this tile's
                        # first S block
                        if kblk == 0 and carry is not None:
                            flush_carry()
                        if prev is not None and kblk == 3:
                            divides_b(*prev)
                            prev = None
                        elif (prev_out and prev is None and 5 <= kblk <= 6
                              and tt < TT - 1):
                            prev_out.pop(0)()
                        elif late and tt >= 6 and not queue and kblk % 2 == 0:
                            late.pop(0)()
                        # the injected pair is needed two tiles out, so
                        # spread pops over this tile's and the next tile's
                        # rounds (dumping everything into a short early
                        # tile stalls the PE on not-yet-landed x DMAs)
                        rounds_left = (nk - 1 - kblk) + (nk + 2)
                        if queue:
                            npop = max(1, -(-len(queue) // rounds_left))
                            for _ in range(min(npop, len(queue))):
                                queue.pop(0)()
                    # leftover prev-tile work that didn't fit this tile
                    if prev is not None:
                        divides_b(*prev)
                        prev = None
                    for fn in prev_out:
                        fn()
                    prev_out = []
                    carry = (tt, ot2, o_quad, list(opend))
                # tail
                for fn in late:
                    fn()
                flush_carry(last=True)
                if prev is not None:
                    divides_b(*prev)
                for fn in prev_out:
                    fn()

    if split:
        _split_waits(nc)
    return nc


def make_in_maps(x, rope_cache, Wqkv, bqkv, Wout, bout):
    """Host-side shard prep. Returns list of 8 in_maps (core = 4*b + g)."""
    x = np.asarray(x, np.float32)
    rope_cache = np.asarray(rope_cache, np.float32)
    Wqkv = np.asarray(Wqkv, np.float32)
    bqkv = np.asarray(bqkv, np.float32)
    Wout = np.asarray(Wout, np.float32)

    # rotary-half permutation within a head: [evens, odds]
    perm = np.concatenate([np.arange(0, D, 2), np.arange(1, D, 2)])
    sin = rope_cache[:, 0::2].T.copy()   # [32, T]
    cos = rope_cache[:, 1::2].T.copy()
    # signed sin for the swap32 rope: rows [-s, +s, -s, +s]; cos tiled 4x
    sinF = np.concatenate([-sin, sin, -sin, sin], axis=0)
    cosF = np.tile(cos, (4, 1))
    scs = np.concatenate([sinF, cosF], axis=1).astype(BF)  # [128, 2T]

    xT = [np.ascontiguousarray(x[b].T).astype(BF) for b in range(B)]

    in_maps = []
    for core in range(N_CORES):
        b, g = divmod(core, G)
        heads = range(HPC * g, HPC * g + HPC)
        # per-head interleave: [A(h0) B(h0) A(h1) B(h1)] for the pp0 tile
        # (heads 0,1 of the core) then the same for pp1 (heads 2,3)
        qcols, kcols, vcols = [], [], []
        for h in heads:
            dd = h * D + perm  # [A(32), B(32)] for this head
            qcols.extend(0 * C + dd)
            kcols.extend(1 * C + dd)
        for h in heads:
            vcols.extend(2 * C + h * D + np.arange(D))
        qcols = np.asarray(qcols)
        kcols = np.asarray(kcols)
        vcols = np.asarray(vcols)
        wq_c = np.ascontiguousarray(Wqkv[:, qcols]).astype(BF)
        wk_c = np.ascontiguousarray(Wqkv[:, kcols]).astype(BF)
        wv_c = np.zeros((C, HPC * VW), np.float32)
        vv = Wqkv[:, vcols]
        for h in range(HPC):
            wv_c[:, VW * h:VW * h + 64] = vv[:, 64 * h:64 * h + 64]
        bqk_c = np.stack([bqkv[qcols[:128]], bqkv[qcols[128:]],
                          bqkv[kcols[:128]], bqkv[kcols[128:]]], axis=1)
        rows = np.arange(HPC * g * D, (HPC * g + HPC) * D)
        wout_c = np.ascontiguousarray(Wout[rows, :]).astype(BF)
        in_maps.append({
            "xT": xT[b], "wq": wq_c, "wk": wk_c,
            "wv": np.ascontiguousarray(wv_c.astype(BF)),
            "bqk": np.ascontiguousarray(bqk_c.astype(np.float32)),
            "scs": scs, "wout": wout_c,
        })
    return in_maps


_NC_CACHE = None


def _get_nc():
    global _NC_CACHE
    if _NC_CACHE is None:
        _NC_CACHE = build_nc()
    return _NC_CACHE


def run(inputs, trace=False):
    nc = _get_nc()
    in_maps = make_in_maps(**inputs)
    res = run_bass_kernel_spmd(nc, in_maps, list(range(N_CORES)), trace=trace)
    Wqkv = np.asarray(inputs["Wqkv"], np.float32)
    bqkv = np.asarray(inputs["bqkv"], np.float32)
    Wout = np.asarray(inputs["Wout"], np.float32)
    bout = np.asarray(inputs["bout"], np.float32)
    bvW = bqkv[2 * C:3 * C] @ Wout            # v-bias through out-proj
    out = np.zeros((B, T, C), np.float32)
    for core in range(N_CORES):
        out[core // G] += np.asarray(res.results[core]["y"], np.float32)
    out += (bvW + bout)[None, None, :]
    return out, res


def kernel(**inputs):
    out, _ = run(inputs)
    return out


# revision 23
# speedup vs baseline: 1.1936x; 1.1936x over previous
"""Causal self-attention (B=2, T=2048, C=1024, H=16, D=64) on 8 NeuronCores.

Sharding: core = (batch b, head-group g); each of the 8 cores handles one
batch and 4 of the 16 heads (data parallel on B, tensor parallel on heads).
Each core computes q/k/v projections for its heads, rope, causal softmax
attention, and a partial out-projection; the host sums the 4 per-batch
partials and adds bout + bqkv_v @ Wout (the v-bias term commutes through
the attention average, so it is applied once on the host).

v8 design (185.5us; v2 baseline was 195.1us):
  - ONE merged exp per k-round over a [P, 2(j), 2(pp), QT] PSUM S tile
    (j = tile_position row half -> bank, pp = head pair -> half bank).
    Halves the ACT instruction count (~350-cycle fixed overhead per
    ACT) and gives the S pool true double buffering (v2 allocated both
    pss bufs every round, serializing S -> exp -> S).
  - O matmuls lag TWO rounds behind S (not one) so they never wait on
    the just-issued exp: the ~1us ACT latency is out of the per-round
    critical cycle. Clean rounds run at ~1.0us (ACT-throughput-paced).
  - x is fully SBUF-resident: pair 0 in fine 512-col chunks (first
    qkproj at ~11us, bounded by the fixed ~7.5us NEFF preamble), pairs
    1-3 in one [P,8,1536] tile split across BOTH DMA queues in the
    prologue. DMA trigger instructions cost ~5ns/descriptor of engine
    time and queues are in-order, so mid-kernel x bursts would delay
    the latency-critical divide DMAs behind megabytes of transfer.
  - steady-state divide: denominator row -> transpose DMA -> [P,8]
    reciprocal -> writeback (latency hidden by the 1-tile pipeline
    slack; tiles 0-3's divides feed late-deferred outprojs so their
    latency is entirely harmless). LAST tile: the round trip sits on
    the tail critical path, so it is done on-engine with two DVE 32x32
    block transposes instead (spread row 64 across partitions 64:96,
    strided 16-elem/lane reciprocal, transpose back).
  - rope runs entirely on DVE (the gpsimd cos-mul made diagonal masks
    queue behind it, stalling O quads ~3us on $S[166]).
  - injected projection work for pair N+1 is paced over the rounds of
    TWO tiles; outproj(12,13) are held back to fill the tail divide
    gap; tail outproj evacuations ride the then-idle ScalarE.
"""
import sys
sys.path.insert(0, '/opt/trn_rl_repo')

import numpy as np
import ml_dtypes
from contextlib import ExitStack

import concourse.bass as bass
import concourse.tile as tile
from concourse import mybir
from concourse.bass_utils import run_bass_kernel_spmd

B, T, C, H, D = 2, 2048, 1024, 16, 64
HPC = 4          # heads per core
G = H // HPC     # head groups (cores per batch)
N_CORES = B * G
SCALE = 1.0 / np.sqrt(D)
P = 128
QT = 256         # q tile width
TT = T // QT     # q tiles (8)
NCC = C // P     # 128-deep contraction chunks (8)
VW = 66          # v cols per head: 64 + ones + pad
F32 = mybir.dt.float32
BF16 = mybir.dt.bfloat16
BF = ml_dtypes.bfloat16


def _tril_mask():
    p = np.arange(P)[:, None]
    f = np.arange(P)[None, :]
    return (p <= f).astype(np.float32)


# walrus in this toolchain can't encode >1 sem wait on one instruction
# ("Too many sync wait commands"); split excess waits onto preceding NoOps.
def _split_waits(nc, maxw=1):
    for f in nc.m.functions:
        for bb in f.blocks:
            out = []
            for inst in bb.instructions:
                si = getattr(inst, 'sync_info', None)
                if si is not None and si.on_wait and len(si.on_wait) > maxw:
                    waits = list(si.on_wait)
                    extra, keep = waits[:-maxw], waits[-maxw:]
                    for i in range(0, len(extra), maxw):
                        out.append(mybir.InstNoOp(
                            name=f"{inst.name}-wsplit{i}",
                            sync_info=mybir.SyncInfo(
                                on_wait=extra[i:i + maxw], on_update=[]),
                            bass_nofuse=True,
                            engine=inst.engine,
                        ))
                    inst.sync_info = mybir.SyncInfo(
                        on_wait=keep, on_update=list(si.on_update or []))
                out.append(inst)
            bb.instructions[:] = out


def build_nc(split=True):
    nc = bass.Bass()
    xT = nc.dram_tensor("xT", [C, T], BF16, kind="ExternalInput")
    wq = nc.dram_tensor("wq", [C, 256], BF16, kind="ExternalInput")
    wk = nc.dram_tensor("wk", [C, 256], BF16, kind="ExternalInput")
    wv = nc.dram_tensor("wv", [C, HPC * VW], BF16, kind="ExternalInput")
    bqk = nc.dram_tensor("bqk", [P, 4], F32, kind="ExternalInput")  # qA qB kA kB
    scs = nc.dram_tensor("scs", [P, 2 * T], BF16, kind="ExternalInput")  # sin|cos
    wout = nc.dram_tensor("wout", [256, C], BF16, kind="ExternalInput")
    y = nc.dram_tensor("y", [T, C], BF16, kind="ExternalOutput")
    masks_d = nc.inline_tensor(_tril_mask(), name="cmasks")

    with tile.TileContext(nc) as tc:
        with ExitStack() as ctx:
            # ---- resident pools ----
            wpool = ctx.enter_context(tc.tile_pool(name="wts", bufs=1))
            qkpool = ctx.enter_context(tc.tile_pool(name="qk", bufs=1))
            vpool = ctx.enter_context(tc.tile_pool(name="v", bufs=1))
            otpool = ctx.enter_context(tc.tile_pool(name="ot", bufs=1))

            bqk_sb = wpool.tile([P, 4], F32, tag="bqk")
            wq_sb = wpool.tile([P, NCC, 256], BF16, tag="wq")
            wk_sb = wpool.tile([P, NCC, 256], BF16, tag="wk")
            wv_sb = wpool.tile([P, NCC, HPC * VW], BF16, tag="wv")
            scs_sb = wpool.tile([P, 2 * T], BF16, tag="scs")
            sin4 = scs_sb[:, 0:T]
            cos4 = scs_sb[:, T:2 * T]
            wout_sb = wpool.tile([P, 2, C], BF16, tag="wout")
            masks_f = wpool.tile([P, P], F32, tag="masksf")
            masks_sb = wpool.tile([P, P], BF16, tag="masks")
            ones_f = wpool.tile([P, 64], F32, tag="onesf")

            # q/k per head pair, rows = [A(h0) B(h0) A(h1) B(h1)] x 32
            # (A/B = rotary low/high halves) so one matmul contracts a
            # whole head (K=64)
            qp0 = qkpool.tile([P, T], BF16, tag="qp0")
            qp1 = qkpool.tile([P, T], BF16, tag="qp1")
            kp0 = qkpool.tile([P, T], BF16, tag="kp0")
            kp1 = qkpool.tile([P, T], BF16, tag="kp1")
            qk_tiles = [qp0, qp1, kp0, kp1]
            qp = [qp0, qp1]
            kp = [kp0, kp1]
            w_of = {0: wq_sb, 1: wq_sb, 2: wk_sb, 3: wk_sb}
            col_of = {0: 0, 1: 128, 2: 0, 3: 128}

            # V tiles [t-block, 4*66] (col 64 per head = ones, 65 = pad)
            NTB = T // P
            v_tiles = [vpool.tile([P, HPC * VW], BF16, tag=f"v{tb}",
                                  name=f"v{tb}")
                       for tb in range(NTB)]

            # O^T in SBUF: heads 0,1 stacked / heads 2,3 stacked
            ot_sb = [otpool.tile([P, T], BF16, tag=f"otsb{i}",
                                 name=f"otsb{i}")
                     for i in range(2)]

            with ExitStack() as stream:
                rtmp = stream.enter_context(tc.tile_pool(name="rtmp", bufs=4))
                # pss tiles are [P, 2(j), 2(pp), QT]: j indexes the PSUM
                # bank so the two concurrent row-group-packed S matmuls
                # (tile_position j=0/1) drain into DIFFERENT banks —
                # concurrent PE drains into one bank are a fatal PSUM
                # write collision on hardware. pp0/pp1 share a bank but
                # those matmuls are serialized (same row group).
                ps_s = stream.enter_context(
                    tc.tile_pool(name="pss", bufs=2, space="PSUM"))
                ps_ot = stream.enter_context(
                    tc.tile_pool(name="psot", bufs=2, space="PSUM"))
                ps_pr = stream.enter_context(
                    tc.tile_pool(name="pspr", bufs=2, space="PSUM"))
                espool = stream.enter_context(tc.tile_pool(name="es", bufs=5))
                dpool = stream.enter_context(tc.tile_pool(name="dv", bufs=2))
                opool = stream.enter_context(tc.tile_pool(name="osb", bufs=2))

                # x pair 0 is loaded in fine 512-col chunks for the
                # fastest possible prologue start; pairs 1-3 live in one
                # resident tile loaded with 8 per-cc DMAs whose DRAM runs
                # are 3KB (descriptor-count, i.e. trigger-engine-time, is
                # 1/3 of per-pair chunked loads), issued once in the
                # prologue so the SP queue has no mid-kernel bursts.
                x0_sb = wpool.tile([P, NCC, 2 * QT], BF16, tag="x0")
                xr_sb = wpool.tile([P, NCC, 3 * 2 * QT], BF16, tag="xr")
                xt = {0: x0_sb}
                for tp in range(1, 4):
                    xt[tp] = xr_sb.rearrange(
                        "p c (u q) -> p c u q", u=3)[:, :, tp - 1, :]

                # q/k projection for one (jb, q-tile-pair): one 8-chunk
                # PSUM generation at N=512; evacuation folds the bias
                # (per-partition) and the bf16 downcast into one DVE op.
                def qkproj(jb, tp):
                    wsb, c0 = w_of[jb], col_of[jb]
                    dst = qk_tiles[jb][:, tp * 2 * QT:(tp + 1) * 2 * QT]
                    ps = ps_pr.tile([P, 512], F32, tag="pspr",
                                    name=f"qk{jb}_{tp}")
                    for cc in range(NCC):
                        nc.tensor.matmul(
                            ps[:], wsb[:, cc, c0:c0 + 128], xt[tp][:, cc, :],
                            start=(cc == 0), stop=(cc == NCC - 1))
                    nc.vector.tensor_scalar_add(dst, ps[:],
                                                bqk_sb[:, jb:jb + 1])

                # rope on the interleaved [A;B]-per-head layout:
                #   t' = t * cosF + swap32(t) * sinF'
                # where swap32 exchanges adjacent 32-partition blocks (DMA)
                # and sinF' = [-sin, +sin, -sin, +sin] (host-built sign).
                def rope(ti, tp):
                    t = qk_tiles[ti]
                    s = slice(tp * 2 * QT, (tp + 1) * 2 * QT)
                    u = rtmp.tile([P, 2 * QT], BF16, tag="ru", name="ru")
                    for blk in range(4):
                        r, rs = 32 * blk, 32 * (blk ^ 1)
                        nc.sync.dma_start(u[r:r + 32, :],
                                          t[rs:rs + 32, s])
                    t1 = rtmp.tile([P, 2 * QT], BF16, tag="r1", name="r1")
                    t2 = rtmp.tile([P, 2 * QT], BF16, tag="r2", name="r2")
                    nc.vector.tensor_mul(t1[:], t[:, s], cos4[:, s])
                    nc.vector.tensor_mul(t2[:], u[:], sin4[:, s])
                    nc.vector.tensor_add(t[:, s], t1[:], t2[:])

                def vproj(tb):
                    ps = ps_pr.tile([P, 512], F32, tag="pspr",
                                    name=f"psv{tb}")[:, 0:HPC * VW]
                    for cc in range(NCC):
                        nc.tensor.matmul(
                            ps, xt[tb // 4][:, cc, (tb % 4) * P:
                                            (tb % 4 + 1) * P],
                            wv_sb[:, cc, :],
                            start=(cc == 0), stop=(cc == NCC - 1))
                    nc.vector.tensor_copy(v_tiles[tb][:], ps)
                    ones_cols = v_tiles[tb].rearrange(
                        "p (h e) -> p h e", e=VW)[:, :, 64]
                    nc.vector.tensor_scalar_add(ones_cols, ones_cols, 1.0)

                # stage 1 of the divide: copy O^T psum (incl. denominator
                # row 64) to SBUF in bf16 — frees the psot banks for the
                # next tt — then the denominator transpose DMA (the [1,512]
                # row must be spread across partitions: DVE reciprocal runs
                # at ~8 cycles/element per lane). x is fully resident after
                # the prologue, so the SP queue has no bursts for these tiny
                # DMAs to queue behind; the last tile rides the then-idle
                # ACT queue.
                def divides_a(tt, ot2, last=False):
                    # The [1,512] denominator row must be spread across
                    # partitions for a cheap reciprocal (DVE recip is ~8
                    # cyc/elem per lane). Steady tiles use a transpose DMA
                    # round-trip (latency hidden by the pipeline); the
                    # LAST tile does it on-engine with two DVE 32x32 block
                    # transposes (~0.6us each vs ~2.5us per DMA leg) since
                    # the round-trip sits on the tail critical path.
                    otf = []
                    if last:
                        rrs = []
                        with nc.allow_low_precision(
                                reason="softmax denom recip in bf16: 0.4% "
                                       "scale error, inside the 2e-2 gate"):
                            for pp in range(2):
                                of = dpool.tile([96, 2, QT], BF16,
                                                tag=f"otfl{pp}",
                                                name=f"otfl{pp}")
                                nc.vector.tensor_copy(of[0:VW], ot2[pp][:])
                                otf.append(of)
                                dnt = dpool.tile([96, 2, QT], BF16,
                                                 tag=f"dnt{pp}", name="dnt")
                                fl = of[64:96, :, :].rearrange(
                                    "p a c -> p (a c)")
                                nc.vector.transpose(
                                    dnt[64:96, :, :].rearrange(
                                        "p a c -> p (a c)"), fl)
                                rw = dpool.tile([96, 2, QT], BF16,
                                                tag=f"rw{pp}", name="rw")
                                nc.vector.reciprocal(
                                    rw[64:96, :, :].rearrange(
                                        "p a c -> p (a c)")[:, 0:512:32],
                                    dnt[64:96, :, :].rearrange(
                                        "p a c -> p (a c)")[:, 0:512:32])
                                rt = dpool.tile([96, 2, QT], BF16,
                                                tag=f"rt{pp}", name="rt")
                                nc.vector.transpose(
                                    rt[64:96, :, :].rearrange(
                                        "p a c -> p (a c)"),
                                    rw[64:96, :, :].rearrange(
                                        "p a c -> p (a c)"))
                                rrs.append(rt[64:65, :, :].rearrange(
                                    "a b c -> a (b c)"))
                        return (otf, rrs, 64)
                    otf = []
                    dn = dpool.tile([P, 8], BF16, tag="dn", name="dn")
                    for pp in range(2):
                        of = dpool.tile([VW, 2, QT], BF16, tag=f"otf{pp}",
                                        name=f"otf{pp}")
                        nc.vector.tensor_copy(of[:], ot2[pp][:])
                        nc.sync.dma_start(
                            dn[:, 4 * pp:4 * pp + 4],
                            of[64:65, :, :].rearrange("a b c -> a (b c)"))
                        otf.append(of)
                    rr = dpool.tile([1, 2, 2, QT], BF16, tag="rr", name="rr")
                    with nc.allow_low_precision(
                            reason="softmax denom recip in bf16: 0.4% scale "
                                   "error, well inside the 2e-2 gate"):
                        dnr = dpool.tile([P, 8], BF16, tag="dnr",
                                         name="dnr")
                        nc.vector.reciprocal(dnr[:], dn[:])
                        for pp in range(2):
                            nc.sync.dma_start(
                                rr[0:1, pp, :, :].rearrange(
                                    "a b c -> a (b c)"),
                                dnr[:, 4 * pp:4 * pp + 4])
                    return (otf,
                            [rr[0:1, pp, :, :].rearrange("a b c -> a (b c)")
                             for pp in range(2)], 0)

                # stage 2: broadcast recip across 64 partitions via PE,
                # then ot_sb[h] = otf[h] * recip (DVE, bf16 2x mode).
                def divides_b(tt, pend):
                    otf, rrs, rbase = pend
                    for pp in range(2):
                        rb = ps_pr.tile([P, 512], F32, tag="pspr",
                                        name=f"rb{pp}")[0:64, :]
                        nc.tensor.matmul(
                            rb, ones_sb[rbase:rbase + 1, :], rrs[pp],
                            start=True, stop=True)
                        rbs = dpool.tile([64, 2, QT], BF16, tag="rbs",
                                         name="rbs")
                        nc.vector.tensor_copy(
                            rbs[:], rb.rearrange("p (a b) -> p a b", a=2))
                        for j in range(2):
                            dst = ot_sb[pp][64 * j:64 * j + 64,
                                            tt * QT:(tt + 1) * QT]
                            nc.vector.tensor_mul(
                                dst, otf[pp][0:64, j, :], rbs[:, j, :])

                def outproj(tb, tail=False):
                    o_sb = opool.tile([P, C], BF16, tag="osb", name="osb")
                    for nt in range(2):
                        ps = ps_pr.tile([P, 512], F32, tag="pspr",
                                        name=f"pso{tb}_{nt}")
                        for rc in range(2):
                            nc.tensor.matmul(
                                ps[:], ot_sb[rc][:, tb * P:(tb + 1) * P],
                                wout_sb[:, rc, nt * 512:(nt + 1) * 512],
                                start=(rc == 0), stop=(rc == 1))
                        # after the last exp the ACT engine is idle; use it
                        # for the tail evacuations so DVE isn't the tail
                        # critical path
                        if tail:
                            nc.scalar.copy(
                                o_sb[:, nt * 512:(nt + 1) * 512], ps[:])
                            nc.sync.dma_start(
                                y[tb * P:(tb + 1) * P,
                                  nt * 512:(nt + 1) * 512],
                                o_sb[:, nt * 512:(nt + 1) * 512])
                        else:
                            nc.vector.tensor_copy(
                                o_sb[:, nt * 512:(nt + 1) * 512], ps[:])
                    if not tail:
                        nc.sync.dma_start(y[tb * P:(tb + 1) * P, :], o_sb[:])

                # ---- prologue ----
                # DMA triggers first (ACT queue), ordered by first use;
                # the ACT table-load dummy exp goes AFTER the triggers so
                # it overlaps the transfers instead of delaying them.
                nc.scalar.dma_start(bqk_sb[:], bqk[:])
                for cc in range(NCC):
                    nc.sync.dma_start(x0_sb[:, cc, :],
                                      xT[cc * P:(cc + 1) * P, 0:2 * QT])
                for cc in range(NCC):
                    nc.scalar.dma_start(wq_sb[:, cc, :],
                                        wq[cc * P:(cc + 1) * P, :])
                for cc in range(NCC):
                    nc.scalar.dma_start(wk_sb[:, cc, :],
                                        wk[cc * P:(cc + 1) * P, :])
                nc.scalar.dma_start(masks_f[:], masks_d[:])
                # scs slices used by rope(tp=0) first, the rest later
                nc.scalar.dma_start(scs_sb[:, 0:2 * QT], scs[:, 0:2 * QT])
                nc.scalar.dma_start(scs_sb[:, T:T + 2 * QT],
                                    scs[:, T:T + 2 * QT])
                nc.scalar.dma_start(wv_sb[:],
                                    wv.rearrange("(o p) n -> p o n", p=P))

                nc.vector.memset(ones_f[:], 1.0)
                ones_sb = wpool.tile([P, 64], BF16, tag="ones")
                nc.vector.tensor_copy(ones_sb[:], ones_f[:])
                # dummy exp: pulls the ~2.7us ACT table load into the DMA
                # prologue
                nc.scalar.activation(ones_f[0:1, 0:1], ones_f[0:1, 0:1],
                                     mybir.ActivationFunctionType.Exp)
                nc.vector.tensor_copy(masks_sb[:], masks_f[:])

                for jb in range(4):
                    qkproj(jb, 0)
                for ti in (0, 2, 1, 3):
                    rope(ti, 0)
                for tb in range(4):
                    vproj(tb)
                # remaining weights + x pair-1 prefetch, behind the
                # first-use loads on their queues
                nc.scalar.dma_start(scs_sb[:, 2 * QT:T], scs[:, 2 * QT:T])
                nc.scalar.dma_start(scs_sb[:, T + 2 * QT:2 * T],
                                    scs[:, T + 2 * QT:2 * T])
                # x pair 1 (needed ~t=20us) is split across BOTH queues
                # so it completes ~8us sooner than either queue alone
                # could deliver it; pairs 2-3 (needed ~t=55/90us) follow
                # on the ACT queue. Sync-queue delay to tiles 0-2's divide
                # DMAs is harmless: those tiles' outprojs are
                # late-deferred. wout goes last (first outproj ~tt=4).
                for cc in range(4):
                    nc.sync.dma_start(
                        xr_sb[:, cc, 0:2 * QT],
                        xT[cc * P:(cc + 1) * P, 2 * QT:4 * QT])
                for cc in range(4, NCC):
                    nc.scalar.dma_start(
                        xr_sb[:, cc, 0:2 * QT],
                        xT[cc * P:(cc + 1) * P, 2 * QT:4 * QT])
                for cc in range(NCC):
                    nc.scalar.dma_start(
                        xr_sb[:, cc, 2 * QT:6 * QT],
                        xT[cc * P:(cc + 1) * P, 4 * QT:T])
                nc.scalar.dma_start(wout_sb[:],
                                    wout.rearrange("(o p) n -> p o n", p=P))

                # ---- streaming attention with injected work ----
                queue = []  # closures of next-tile + prev-tile work
                prev = None
                prev_out = []  # outproj closures of prev tile not yet run
                late = []  # early-tile outprojs deferred into the last tiles
                carry = None  # prev tile's deferred final O flush

                def flush_carry(last=False):
                    # the previous tile's last two O rounds, deferred past
                    # the next tile's first S so they never wait on exp
                    nonlocal carry, prev, prev_out
                    ptt, pot2, pfn, popend = carry
                    carry = None
                    for i, pnd in enumerate(popend):
                        pfn(*pnd, stop=(i == len(popend) - 1))
                    if prev is not None:
                        divides_b(*prev)
                        prev = None
                    if last:
                        pend = divides_a(ptt, pot2, last=True)
                        for tb in (2 * ptt - 2, 2 * ptt - 1):
                            outproj(tb, tail=True)
                        prev_out = [lambda tb=tb: outproj(tb, tail=True)
                                    for tb in range(2 * ptt, 2 * ptt + 2)]
                        prev = (ptt, pend)
                        return
                    for fn in prev_out:
                        fn()
                    outs = [lambda tb=tb, tl=last: outproj(tb, tail=tl)
                            for tb in range(2 * ptt, 2 * ptt + 2)]
                    if ptt < 4:
                        late.extend(outs)
                        prev_out = []
                    else:
                        prev_out = outs
                    prev = (ptt, divides_a(ptt, pot2, last=last))

                for tt in range(TT):
                    nk = 2 * tt + 2
                    if tt % 2 == 0 and tt // 2 + 1 < TT // 2:
                        ntp = tt // 2 + 1
                        queue += [lambda jb=jb, t=ntp: qkproj(jb, t)
                                  for jb in range(4)]
                        queue += [lambda ti=ti, t=ntp: rope(ti, t)
                                  for ti in (0, 2, 1, 3)]
                        queue += [lambda tb=tb: vproj(tb)
                                  for tb in range(4 * ntp, 4 * ntp + 4)]
                    ot2 = [ps_ot.tile([VW, 2, QT], F32, tag="psot",
                                      name=f"psot{pp}") for pp in range(2)]
                    opend = []  # rounds whose O quad hasn't been emitted

                    def o_quad(pes, poff, pk, stop=False, ot2=ot2):
                        for pp in range(2):
                            for j in range(2):
                                h = 2 * pp + j
                                nc.tensor.matmul(
                                    ot2[pp][:, j, poff:],
                                    v_tiles[pk][:, VW * h:VW * h + VW],
                                    pes[:, j, pp, poff:],
                                    start=(pk == 0 and j == 0),
                                    stop=(stop and j == 1))

                    for kblk in range(nk):
                        off = max(0, (kblk - 2 * tt)) * P
                        ks = slice(kblk * P, (kblk + 1) * P)
                        qs = slice(tt * QT + off, (tt + 1) * QT)
                        s4 = ps_s.tile([P, 2, 2, QT], F32, tag="pss",
                                       name="pss")
                        for pp in range(2):
                            for j in range(2):
                                hs = slice(64 * j, 64 * j + 64)
                                nc.tensor.matmul(
                                    s4[:, j, pp, off:], kp[pp][hs, ks],
                                    qp[pp][hs, qs],
                                    start=True, stop=True,
                                    tile_position=(64 * j, 0))
                        es4 = espool.tile([P, 2, 2, QT], BF16, tag="es",
                                          name="es")
                        nc.scalar.activation(
                            es4[:, :, :, off:], s4[:, :, :, off:],
                            mybir.ActivationFunctionType.Exp, scale=SCALE)
                        if kblk >= 2 * tt:
                            for pp in range(2):
                                nc.gpsimd.tensor_mul(
                                    es4[:, :, pp, off:off + P],
                                    es4[:, :, pp, off:off + P],
                                    masks_sb[:, None, :].to_broadcast(
                                        (P, 2, P)))
                        # O lags TWO rounds behind S so it never waits on
                        # the just-issued exp (the 1-round lag put exp's
                        # ~1us latency inside the per-round critical cycle)
                        opend.append((es4, off, kblk))
                        if len(opend) > 2:
                            o_quad(*opend.pop(0))
                        # deferred prev-tile flush right after this tile's
                        # first S block
                        if kblk == 0 and carry is not None:
                            flush_carry()
                        if prev is not None and kblk == 3:
                            divides_b(*prev)
                            prev = None
                        elif (prev_out and prev is None and 5 <= kblk <= 6
                              and tt < TT - 1):
                            prev_out.pop(0)()
                        elif late and tt >= 6 and not queue and kblk % 2 == 0:
                            late.pop(0)()
                        # the injected pair is needed two tiles out, so
                        # spread pops over this tile's and the next tile's
                        # rounds (dumping everything into a short early
                        # tile stalls the PE on not-yet-landed x DMAs)
                        rounds_left = (nk - 1 - kblk) + (nk + 2)
                        if queue:
                            npop = max(1, -(-len(queue) // rounds_left))
                            for _ in range(min(npop, len(queue))):
                                queue.pop(0)()
                    # leftover prev-tile work that didn't fit this tile
                    if prev is not None:
                        divides_b(*prev)
                        prev = None
                    for fn in prev_out:
                        fn()
                    prev_out = []
                    carry = (tt, ot2, o_quad, list(opend))
                # tail
                for fn in late:
                    fn()
                flush_carry(last=True)
                if prev is not None:
                    divides_b(*prev)
                for fn in prev_out:
                    fn()

    if split:
        _split_waits(nc)
    return nc


def make_in_maps(x, rope_cache, Wqkv, bqkv, Wout, bout):
    """Host-side shard prep. Returns list of 8 in_maps (core = 4*b + g)."""
    x = np.asarray(x, np.float32)
    rope_cache = np.asarray(rope_cache, np.float32)
    Wqkv = np.asarray(Wqkv, np.float32)
    bqkv = np.asarray(bqkv, np.float32)
    Wout = np.asarray(Wout, np.float32)

    # rotary-half permutation within a head: [evens, odds]
    perm = np.concatenate([np.arange(0, D, 2), np.arange(1, D, 2)])
    sin = rope_cache[:, 0::2].T.copy()   # [32, T]
    cos = rope_cache[:, 1::2].T.copy()
    # signed sin for the swap32 rope: rows [-s, +s, -s, +s]; cos tiled 4x
    sinF = np.concatenate([-sin, sin, -sin, sin], axis=0)
    cosF = np.tile(cos, (4, 1))
    scs = np.concatenate([sinF, cosF], axis=1).astype(BF)  # [128, 2T]

    xT = [np.ascontiguousarray(x[b].T).astype(BF) for b in range(B)]

    in_maps = []
    for core in range(N_CORES):
        b, g = divmod(core, G)
        heads = range(HPC * g, HPC * g + HPC)
        # per-head interleave: [A(h0) B(h0) A(h1) B(h1)] for the pp0 tile
        # (heads 0,1 of the core) then the same for pp1 (heads 2,3)
        qcols, kcols, vcols = [], [], []
        for h in heads:
            dd = h * D + perm  # [A(32), B(32)] for this head
            qcols.extend(0 * C + dd)
            kcols.extend(1 * C + dd)
        for h in heads:
            vcols.extend(2 * C + h * D + np.arange(D))
        qcols = np.asarray(qcols)
        kcols = np.asarray(kcols)
        vcols = np.asarray(vcols)
        wq_c = np.ascontiguousarray(Wqkv[:, qcols]).astype(BF)
        wk_c = np.ascontiguousarray(Wqkv[:, kcols]).astype(BF)
        wv_c = np.zeros((C, HPC * VW), np.float32)
        vv = Wqkv[:, vcols]
        for h in range(HPC):
            wv_c[:, VW * h:VW * h + 64] = vv[:, 64 * h:64 * h + 64]
        bqk_c = np.stack([bqkv[qcols[:128]], bqkv[qcols[128:]],
                          bqkv[kcols[:128]], bqkv[kcols[128:]]], axis=1)
        rows = np.arange(HPC * g * D, (HPC * g + HPC) * D)
        wout_c = np.ascontiguousarray(Wout[rows, :]).astype(BF)
        in_maps.append({
            "xT": xT[b], "wq": wq_c, "wk": wk_c,
            "wv": np.ascontiguousarray(wv_c.astype(BF)),
            "bqk": np.ascontiguousarray(bqk_c.astype(np.float32)),
            "scs": scs, "wout": wout_c,
        })
    return in_maps


_NC_CACHE = None


def _get_nc():
    global _NC_CACHE
    if _NC_CACHE is None:
        _NC_CACHE = build_nc()
    return _NC_CACHE


def run(inputs, trace=False):
    nc = _get_nc()
    in_maps = make_in_maps(**inputs)
    res = run_bass_kernel_spmd(nc, in_maps, list(range(N_CORES)), trace=trace)
    Wqkv = np.asarray(inputs["Wqkv"], np.float32)
    bqkv = np.asarray(inputs["bqkv"], np.float32)
    Wout = np.asarray(inputs["Wout"], np.float32)
    bout = np.asarray(inputs["bout"], np.float32)
    bvW = bqkv[2 * C:3 * C] @ Wout            # v-bias through out-proj
    out = np.zeros((B, T, C), np.float32)
    for core in range(N_CORES):
        out[core // G] += np.asarray(res.results[core]["y"], np.float32)
    out += (bvW + bout)[None, None, :]
    return out, res


def kernel(**inputs):
    out, _ = run(inputs)
    return out
